# revision 16
# baseline (speedup 1.0000x reference)
"""Trainium2 Bass kernel for 16-head MHA (d_model=1024, batch 4, seq 2048).

Sharding: batch (4) x head-group (2) across 8 NeuronCores. Each core computes
one batch sample's attention for 8 of the 16 heads plus its partial output
projection; the host sums the two partial outputs per sample and adds the
bias terms.

Per-core dataflow (all matmul contractions run on the partition axis):
  q^T/k^T = WqT'.T @ x^T   (fp16, out [o, t] with heads on partitions)
  v       = x^T.T @ WvT    (fp16 matmul, bf16 store, [t, o] with a ones column
                            per head for fused softmax rowsums)
  E^T     = exp(kT_h.T @ qT_h)  (scores f32 in PSUM, exp on ACT, bf16 store;
                                 no max-subtraction: reference applies no
                                 1/sqrt(d) scaling and scores stay < ~50)
  att_h   = (V_h|1).T @ E^T_h   -> rows 0:64 raw attention, row 64 rowsum
  att^T   = att_h * recip(rowsum)
  y^T     = WoT'.T @ att^T      (bf16 out, host sums the two partials in f32)

The run is ACT(exp)-limited in steady state: 256 exps of [128,1024] at
(1024+352)/1.2 ns each = 293us. The schedule keeps ACT saturated:

- The two heads of a pair run their score matmuls CONCURRENTLY as row-tiled
  MMs (K=64 each, row groups 0/64) into one fused [128, 2*512] PSUM tile, so
  one exp covers the pair and s1 PE time halves vs per-head issue.
- Attention runs as ONE flat software pipeline over 16 blocks of (head-pair x
  query-quarter) x 16 key chunks; att@V trails the exp by 2 chunks and flows
  across block boundaries with no drain.
- Projections use N=1024 moving operands (fp16 allows it), halving matmul and
  ldweights count, and are dripped into the kc loops as ~1-matmul microtasks
  in data-arrival order (filler emitted before its consumer block: Tile deps
  only point backward in program order).
- Inputs stream in need-order over three DMA queues with >=1KB descriptor
  rows (512B-row chunking measured at only 31 GB/s):
    sync:   xv half-slabs (2KB rows) -> v proj chase, then xq qq0/1 chunks
    scalar: wk + xk full rows (4KB)  -> k proj chase
    gpsimd: wv, wq, biases, xq qq2/3, Wo
  Prologue PE order chases arrivals: v proj, q(hp0) first half, k(hp0).
- The tail emits y(qq3) with the hp3-dependent oc last, so most of the final
  projection overlaps the last block's normalize.

fp16 is used for the whole q/k/scores path: bf16's 8-bit mantissa gives score
errors ~0.05 which exp() amplifies to ~2e-2 output error; fp16 keeps it ~3e-3.
"""

from collections import deque
from contextlib import ExitStack

import numpy as np

import concourse.bacc as bacc
import concourse.mybir as mybir
import concourse.tile as tile
from concourse.bass_utils import run_bass_kernel_spmd

F32 = mybir.dt.float32
F16 = mybir.dt.float16
BF16 = mybir.dt.bfloat16

D = 1024          # d_model
HD = 64           # head dim
NH_CORE = 8       # heads per core
OC = NH_CORE * HD # per-core q/k/v output dims (512)
N_CORES = 8
NI = D // 128     # contraction chunks for projections
NOC = OC // 128   # o-chunks (head pairs)
NDC = D // 128    # output-dim chunks for the final projection


def build_kernel(S=2048):
    nc = bacc.Bacc("TRN2", target_bir_lowering=False, debug=False)

    xq_d = nc.dram_tensor("xqT", (D, S), F16, kind="ExternalInput")
    xk_d = nc.dram_tensor("xkT", (D, S), F16, kind="ExternalInput")
    xv_d = nc.dram_tensor("xvT", (D, S), F16, kind="ExternalInput")
    wq_d = nc.dram_tensor("WqT", (D, OC), F16, kind="ExternalInput")
    wk_d = nc.dram_tensor("WkT", (D, OC), F16, kind="ExternalInput")
    wv_d = nc.dram_tensor("WvT", (D, OC), F16, kind="ExternalInput")
    wo_d = nc.dram_tensor("WoT", (OC, D), F16, kind="ExternalInput")
    bq_d = nc.dram_tensor("bq", (OC,), F32, kind="ExternalInput")
    bk_d = nc.dram_tensor("bk", (OC,), F32, kind="ExternalInput")
    y_d = nc.dram_tensor("yT", (D, S), BF16, kind="ExternalOutput")

    NKC = S // 128            # key chunks (16)
    QQ = 512                  # query-quarter size (s1 moving width per head)
    NQQ = S // QQ             # 4
    SH = S // 2               # xv half-slab width

    Exp = mybir.ActivationFunctionType.Exp
    Mult = mybir.AluOpType.mult

    with tile.TileContext(nc) as tc, ExitStack() as ctx:
        wpool = ctx.enter_context(tc.tile_pool(name="w", bufs=1))
        xpool = ctx.enter_context(tc.tile_pool(name="x", bufs=1))
        spool = ctx.enter_context(tc.tile_pool(name="seq", bufs=1))
        epool = ctx.enter_context(tc.tile_pool(name="e", bufs=3))
        evpool = ctx.enter_context(tc.tile_pool(name="ev", bufs=2))
        npool = ctx.enter_context(tc.tile_pool(name="nrm", bufs=2))
        pjpool = ctx.enter_context(tc.tile_pool(name="pj", bufs=1, space="PSUM"))
        s1pool = ctx.enter_context(tc.tile_pool(name="s1", bufs=2, space="PSUM"))
        accpool = ctx.enter_context(tc.tile_pool(name="acc", bufs=2, space="PSUM"))

        # ---- resident weights / biases / x slabs ----
        wq_sb = wpool.tile([128, NI, OC], F16, tag="wq")
        wk_sb = wpool.tile([128, NI, OC], F16, tag="wk")
        wv_sb = wpool.tile([128, NI, OC], F16, tag="wv")
        wo_sb = wpool.tile([128, NOC, D], F16, tag="wo")
        bq_sb = wpool.tile([128, NOC], F32, tag="bq")
        bk_sb = wpool.tile([128, NOC], F32, tag="bk")

        xq_sb = xpool.tile([128, NI, S], F16, tag="xq")
        xk_sb = xpool.tile([128, NI, S], F16, tag="xk")
        # xv streams through a 2-deep ring of quarter-slabs (512 t each);
        # the ring throttles the sync queue to a bounded lookahead and the
        # v-proj subunits consume quarters in order
        xv_sb = [xpool.tile([128, NI, S // 4], F16, tag="xv", bufs=2,
                            name=f"xvq{vq}") for vq in range(4)]

        # sync HW queue: xv half-slabs (2KB rows, v-proj chase), then xq
        # quarters 0/1 interleaved ic-major (q(hp0) chase)
        QV = S // 4
        for vq in (0, 1):
            for ic in range(NI):
                nc.sync.dma_start(
                    out=xv_sb[vq][:, ic, :],
                    in_=xv_d.ap()[ic * 128:(ic + 1) * 128, vq * QV:(vq + 1) * QV],
                )
        for ic in range(NI):
            nc.sync.dma_start(
                out=xq_sb[:, ic, 0:2 * QQ],
                in_=xq_d.ap()[ic * 128:(ic + 1) * 128, 0:2 * QQ],
            )
        for vq in (2, 3):
            for ic in range(NI):
                nc.sync.dma_start(
                    out=xv_sb[vq][:, ic, :],
                    in_=xv_d.ap()[ic * 128:(ic + 1) * 128, vq * QV:(vq + 1) * QV],
                )
        # scalar HW queue: wk + xk full rows (4KB rows, k-proj chase)
        nc.scalar.dma_start(out=wk_sb, in_=wk_d.ap().rearrange("(ic p) o -> p ic o", p=128))
        for ic in range(NI):
            nc.scalar.dma_start(out=xk_sb[:, ic, :],
                                in_=xk_d.ap()[ic * 128:(ic + 1) * 128, :])
        # gpsimd SW queue: wv (first v matmul ~4us), wq, biases, xq 2/3, Wo
        nc.gpsimd.dma_start(out=wv_sb, in_=wv_d.ap().rearrange("(ic p) o -> p ic o", p=128))
        nc.gpsimd.dma_start(out=wq_sb, in_=wq_d.ap().rearrange("(ic p) o -> p ic o", p=128))
        nc.gpsimd.dma_start(out=bq_sb, in_=bq_d.ap().rearrange("(c p) -> p c", p=128))
        nc.gpsimd.dma_start(out=bk_sb, in_=bk_d.ap().rearrange("(c p) -> p c", p=128))
        for ic in range(NI):
            nc.gpsimd.dma_start(
                out=xq_sb[:, ic, 2 * QQ:4 * QQ],
                in_=xq_d.ap()[ic * 128:(ic + 1) * 128, 2 * QQ:4 * QQ],
            )
        nc.gpsimd.dma_start(out=wo_sb, in_=wo_d.ap().rearrange("(oc p) d -> p oc d", p=128))

        # ---- per-sequence slabs ----
        v_sb = spool.tile([128, NKC, NH_CORE * (HD + 1)], BF16, tag="v")
        qT_sb = spool.tile([128, NOC, S], F16, tag="qT")
        kT_sb = spool.tile([128, NOC, S], F16, tag="kT")
        # bf16 att: the y projection's N=1024 moving operand requires a
        # 16-bit dtype with 1024-wide ISA support (fp16 caps at 512)
        att_sb = spool.tile([128, NOC, S], BF16, tag="att")

        # ---- microtask unit factories (units are atomic: each holds the
        # ---- single [128,1024] pj psum slot until its trailing DVE op) ----
        def v_sub(vq, tp):
            """v projection for two key chunks (256 t): 16 MMs + copy."""
            kc0 = vq * 4 + tp * 2
            state = {}
            def mk_mm(ic):
                def f():
                    if "pj" not in state:
                        state["pj"] = pjpool.tile([128, 2 * OC], F32, tag="pj",
                                                  name=f"vps{vq}_{tp}")
                    for j in range(2):
                        nc.tensor.matmul(
                            state["pj"][:, j * OC:(j + 1) * OC],
                            xv_sb[vq][:, ic, (tp * 2 + j) * 128:(tp * 2 + j + 1) * 128],
                            wv_sb[:, ic, :],
                            start=(ic == 0), stop=(ic == NI - 1),
                        )
                return f
            def fin():
                for j in range(2):
                    vv = v_sb[:, kc0 + j, :].rearrange("p (h c) -> p h c", h=NH_CORE)
                    nc.vector.tensor_copy(
                        out=vv[:, :, 0:HD],
                        in_=state["pj"][:, j * OC:(j + 1) * OC]
                            .rearrange("p (h c) -> p h c", c=HD),
                    )
                    nc.vector.memset(vv[:, :, HD:HD + 1], 1.0)
            return [mk_mm(ic) for ic in range(NI)] + [fin]

        def qk_unit(w_sb, x_sb, b_sb, dst, hp, half, label):
            """q/k projection for two quarters (fp16 moving caps at N=512,
            so two MMs per stationary into the wide pj tile's halves)."""
            state = {}
            def mk_mm(ic):
                def f():
                    if "pj" not in state:
                        state["pj"] = pjpool.tile([128, 2 * QQ], F32, tag="pj",
                                                  name=f"pj{label}{hp}_{half}")
                    for j in range(2):
                        q0 = (half * 2 + j) * QQ
                        nc.tensor.matmul(
                            state["pj"][:, j * QQ:(j + 1) * QQ],
                            w_sb[:, ic, hp * 128:(hp + 1) * 128],
                            x_sb[:, ic, q0:q0 + QQ],
                            start=(ic == 0), stop=(ic == NI - 1),
                        )
                return f
            def bias():
                nc.vector.tensor_scalar_add(
                    out=dst[:, hp, half * 2 * QQ:(half + 1) * 2 * QQ],
                    in0=state["pj"][:, :],
                    scalar1=b_sb[:, hp:hp + 1],
                )
            return [mk_mm(ic) for ic in range(NI)] + [bias]

        def q_quarter(hp, qq):
            """q projection for a single 512-query quarter (first-use path)."""
            state = {}
            def mk_mm(icp):
                def f():
                    if "pj" not in state:
                        state["pj"] = pjpool.tile([128, 2 * QQ], F32, tag="pj",
                                                  name=f"pjq{hp}_{qq}")
                    for ic in (2 * icp, 2 * icp + 1):
                        nc.tensor.matmul(
                            state["pj"][:, 0:QQ],
                            wq_sb[:, ic, hp * 128:(hp + 1) * 128],
                            xq_sb[:, ic, qq * QQ:(qq + 1) * QQ],
                            start=(ic == 0), stop=(ic == NI - 1),
                        )
                return f
            def bias():
                nc.vector.tensor_scalar_add(
                    out=qT_sb[:, hp, qq * QQ:(qq + 1) * QQ],
                    in0=state["pj"][:, 0:QQ],
                    scalar1=bq_sb[:, hp:hp + 1],
                )
            return [mk_mm(i) for i in range(NI // 2)] + [bias]

        def y_unit(dc, q0, qw, oc_mts, name):
            """output projection for [128 d rows, qw queries] (qw <= 1024 as
            1-2 N=512 MMs per stationary): oc-grouped MM microtasks (so the
            hp3-dependent oc can go last) + bf16 store."""
            nj = qw // QQ
            state = {}
            def mk_mm(ocs):
                def f():
                    if "pj" not in state:
                        state["pj"] = pjpool.tile([128, 2 * QQ], F32, tag="pj",
                                                  name=f"yps{name}_{dc}")
                    for oc in ocs:
                        for j in range(nj):
                            nc.tensor.matmul(
                                state["pj"][:, j * QQ:(j + 1) * QQ],
                                wo_sb[:, oc, dc * 128:(dc + 1) * 128],
                                att_sb[:, oc, q0 + j * QQ:q0 + (j + 1) * QQ],
                                start=(oc == 0), stop=(oc == NOC - 1),
                            )
                return f
            def store():
                y_sb = evpool.tile([128, 2 * QQ], BF16, tag="yev",
                                   name=f"yev{name}_{dc}")
                nc.vector.tensor_copy(out=y_sb[:, 0:qw], in_=state["pj"][:, 0:qw])
                nc.sync.dma_start(
                    out=y_d.ap()[dc * 128:(dc + 1) * 128, q0:q0 + qw],
                    in_=y_sb[:, 0:qw],
                )
            return [mk_mm(ocs) for ocs in oc_mts] + [store]

        # ---- filler schedule ----
        fillers = deque()

        def pull(n=1):
            for _ in range(n):
                if fillers:
                    fillers.popleft()()

        def run_unit(u):
            for f in u:
                f()

        # ---- prologue: chase DMA arrivals ----
        # xv q0/q1 -> v subs; xq qq0/1 -> q(0,A); xv q2/q3 -> v subs;
        # xk -> k(0,A/B).  First s1 issues right after k(0,B).
        for vq in (0, 1):
            for tp in range(2):
                run_unit(v_sub(vq, tp))
        run_unit(qk_unit(wq_sb, xq_sb, bq_sb, qT_sb, 0, 0, "q"))
        for vq in (2, 3):
            for tp in range(2):
                run_unit(v_sub(vq, tp))
        run_unit(qk_unit(wk_sb, xk_sb, bk_sb, kT_sb, 0, 0, "k"))
        run_unit(qk_unit(wk_sb, xk_sb, bk_sb, kT_sb, 0, 1, "k"))

        # ---- filler population in need-order ----
        # CORRECTNESS: a filler feeding block N must be fully emitted before
        # block N's first reader.  Block starts (16 pulls each): b2=32 needs
        # k(1)+q(1,qq0) -> 23 mts; deadlines loosen from there.
        for hp in (1, 2, 3):
            fillers.extend(qk_unit(wk_sb, xk_sb, bk_sb, kT_sb, hp, 0, "k"))
            fillers.extend(qk_unit(wk_sb, xk_sb, bk_sb, kT_sb, hp, 1, "k"))
            fillers.extend(q_quarter(hp, 0))
            fillers.extend(q_quarter(hp, 1))
        fillers.extend(qk_unit(wq_sb, xq_sb, bq_sb, qT_sb, 0, 1, "q"))

        later_q = []
        for hp in (1, 2, 3):
            later_q.append(qk_unit(wq_sb, xq_sb, bq_sb, qT_sb, hp, 1, "q"))

        # block order: quarters 0/1 per pair first (k/q prefetch lead), then
        # quarters 2/3 swept by-quarter so only y(qq3) tails
        order = ([(hp, qq) for hp in range(NOC) for qq in (0, 1)]
                 + [(hp, 2) for hp in range(NOC)]
                 + [(hp, 3) for hp in range(NOC)])

        # ---- flat attention pipeline ----
        pend = deque()
        accs_by_bi = {}
        qq_done = {qq: 0 for qq in range(NQQ)}

        def finalize(bi, hp, qq):
            # normalize straight from the acc PSUM banks: the DVE mult (the
            # acc's last reader) lands well before the next block's first
            # s2 needs the ring slot, so no SBUF staging copy is needed
            qoff = qq * QQ
            accs = accs_by_bi.pop(bi)
            for hl in range(2):
                off = hl * 64
                rt = npool.tile([1, QQ], F32, tag="rtmp", bufs=2,
                                name=f"rt{bi}_{hl}")
                nc.vector.tensor_copy(out=rt[:, :], in_=accs[hl][64:65, :])
                nc.vector.reciprocal_approx_fast(out=rt[:, :], in_=rt[:, :])
                bc = npool.tile([64, QQ], F32, tag="bcast", bufs=2,
                                name=f"bc{bi}_{hl}")
                nc.gpsimd.partition_broadcast(out_ap=bc[:, :], in_ap=rt[:, :])
                nc.vector.tensor_tensor(
                    out=att_sb[off:off + 64, hp, qoff:qoff + QQ],
                    in0=accs[hl][0:64, :],
                    in1=bc[:, :],
                    op=Mult,
                )
            qq_done[qq] += 1
            if qq_done[qq] == NOC:
                if qq == 1:
                    # quarters 0+1 done: y over N=1024 (halved ldweights)
                    for dc in range(NDC):
                        fillers.extend(
                            y_unit(dc, 0, 2 * QQ, [(0, 1), (2, 3)], "y01"))
                elif qq == 2:
                    for dc in range(NDC):
                        fillers.extend(
                            y_unit(dc, 2 * QQ, QQ, [(0, 1), (2, 3)], "y2"))
                # qq3 is emitted in the tail with oc3 last (see below)

        def s2_pop():
            bi2, hp2, qq2, e2, kc2 = pend.popleft()
            if kc2 == 0:
                accs_by_bi[bi2] = [
                    accpool.tile([65, QQ], F32, tag="acc", name=f"acc{bi2}_{hl}")
                    for hl in range(2)
                ]
            accs = accs_by_bi[bi2]
            for hl in range(2):
                h = 2 * hp2 + hl
                nc.tensor.matmul(
                    accs[hl][:, :],
                    v_sb[:, kc2, h * (HD + 1):(h + 1) * (HD + 1)],
                    e2[:, hl * QQ:(hl + 1) * QQ],
                    start=(kc2 == 0), stop=(kc2 == NKC - 1),
                )
            if kc2 == NKC - 1:
                finalize(bi2, hp2, qq2)

        for bi, (hp, qq) in enumerate(order):
            if bi == 6:
                for u in later_q:
                    fillers.extend(u)
            qoff = qq * QQ
            for kc in range(NKC):
                s1 = s1pool.tile([128, 2 * QQ], F32, tag="s1",
                                 name=f"s1_{bi}_{kc}")
                # the two heads' score MMs land on row groups 0/64 and
                # stream concurrently; one exp covers the fused tile
                for hl in range(2):
                    off = hl * 64
                    nc.tensor.matmul(
                        s1[:, hl * QQ:(hl + 1) * QQ],
                        kT_sb[off:off + 64, hp, kc * 128:(kc + 1) * 128],
                        qT_sb[off:off + 64, hp, qoff:qoff + QQ],
                        start=True, stop=True,
                    )
                e = epool.tile([128, 2 * QQ], BF16, tag="e", name=f"e{bi}_{kc}")
                nc.scalar.activation(out=e[:, :], in_=s1[:, :], func=Exp)
                pend.append((bi, hp, qq, e, kc))
                if len(pend) > 2:
                    s2_pop()
                pull(1)
        while pend:
            s2_pop()

        # tail: y(qq3) with the hp3-dependent oc last so most of the
        # projection overlaps the final block's normalize
        for dc in range(NDC):
            run_unit(y_unit(dc, 3 * QQ, QQ, [(0, 1), (2,), (3,)], "y3"))
        while fillers:
            fillers.popleft()()

    nc.compile()
    return nc


def make_in_maps(query, key, value, Wq, bq, Wk, bk, Wv, bv, Wo, bo):
    """Shard + lay out full inputs for the 8 cores: core = 2*n + g."""
    f16 = np.float16
    N = query.shape[0]
    per_g = {}
    for g in range(2):
        osl = slice(g * OC, (g + 1) * OC)
        per_g[g] = dict(
            WqT=np.ascontiguousarray(Wq[osl, :].T).astype(f16),
            WkT=np.ascontiguousarray(Wk[osl, :].T).astype(f16),
            WvT=np.ascontiguousarray(Wv[osl, :].T).astype(f16),
            WoT=np.ascontiguousarray(Wo[:, osl].T).astype(f16),
            bq=np.ascontiguousarray(bq[osl]).astype(np.float32),
            bk=np.ascontiguousarray(bk[osl]).astype(np.float32),
        )
    in_maps = []
    for n in range(N):
        xqT = np.ascontiguousarray(query[n].T).astype(f16)
        xkT = np.ascontiguousarray(key[n].T).astype(f16)
        xvT = np.ascontiguousarray(value[n].T).astype(f16)
        for g in range(2):
            m = dict(xqT=xqT, xkT=xkT, xvT=xvT)
            m.update(per_g[g])
            in_maps.append(m)
    return in_maps


_BUILT = None


def _get_built():
    global _BUILT
    if _BUILT is None:
        _BUILT = build_kernel(2048)
    return _BUILT


def kernel(query, key, value, Wq, bq, Wk, bk, Wv, bv, Wo, bo, _results=None):
    query = np.asarray(query, np.float32)
    key = np.asarray(key, np.float32)
    value = np.asarray(value, np.float32)
    Wq, bq = np.asarray(Wq, np.float32), np.asarray(bq, np.float32)
    Wk, bk = np.asarray(Wk, np.float32), np.asarray(bk, np.float32)
    Wv, bv = np.asarray(Wv, np.float32), np.asarray(bv, np.float32)
    Wo, bo = np.asarray(Wo, np.float32), np.asarray(bo, np.float32)

    N, S, _ = query.shape
    if _results is None:
        nc = _get_built()
        in_maps = make_in_maps(query, key, value, Wq, bq, Wk, bk, Wv, bv, Wo, bo)
        res = run_bass_kernel_spmd(nc, in_maps, list(range(N_CORES)))
        _results = res.results

    const = bv @ Wo.T + bo  # host-folded bias terms
    out = np.empty((N, S, D), np.float32)
    for n in range(N):
        yT = (_results[2 * n]["yT"].astype(np.float32)
              + _results[2 * n + 1]["yT"].astype(np.float32))
        out[n] = yT.T + const
    return out


# revision 22
# speedup vs baseline: 1.0650x; 1.0650x over previous
"""Trainium2 Bass kernel for 16-head MHA (d_model=1024, batch 4, seq 2048).

Sharding: batch (4) x head-group (2) across 8 NeuronCores. Each core computes
one batch sample's attention for 8 of the 16 heads plus its partial output
projection; the host sums the two partial outputs per sample and adds the
bias terms.

Per-core dataflow (all matmul contractions run on the partition axis):
  q^T/k^T = WqT'.T @ x^T   (fp16, out [o, t] with heads on partitions)
  v       = x^T.T @ WvT    (fp16 matmul, bf16 store, [t, o] with a ones column
                            per head for fused softmax rowsums)
  E^T     = exp(kT_h.T @ qT_h)  (scores f32 in PSUM, exp on ACT, bf16 store;
                                 no max-subtraction: reference applies no
                                 1/sqrt(d) scaling and scores stay < ~50)
  att_h   = (V_h|1).T @ E^T_h   -> rows 0:64 raw attention, row 64 rowsum
  att^T   = att_h * recip(rowsum)
  y^T     = WoT'.T @ att^T      (bf16 out, host sums the two partials in f32)

The run is ACT(exp)-limited in steady state: 256 exps of [128,1024] at
(1024+352)/1.2 ns each = 293us. The schedule keeps ACT saturated:

- The two heads of a pair run their score matmuls CONCURRENTLY as row-tiled
  MMs (K=64 each, row groups 0/64) into one fused [128, 2*512] PSUM tile, so
  one exp covers the pair and s1 PE time halves vs per-head issue.
- Attention runs as ONE flat software pipeline over 16 blocks of (head-pair x
  query-quarter) x 16 key chunks; att@V trails the exp by 2 chunks and flows
  across block boundaries with no drain.
- Projections use N=1024 moving operands (fp16 allows it), halving matmul and
  ldweights count, and are dripped into the kc loops as ~1-matmul microtasks
  in data-arrival order (filler emitted before its consumer block: Tile deps
  only point backward in program order).
- Inputs stream in need-order over three DMA queues with >=1KB descriptor
  rows (512B-row chunking measured at only 31 GB/s):
    sync:   xv half-slabs (2KB rows) -> v proj chase, then xq qq0/1 chunks
    scalar: wk + xk full rows (4KB)  -> k proj chase
    gpsimd: wv, wq, biases, xq qq2/3, Wo
  Prologue PE order chases arrivals: v proj, q(hp0) first half, k(hp0).
- The tail emits y(qq3) with the hp3-dependent oc last, so most of the final
  projection overlaps the last block's normalize.

fp16 is used for the whole q/k/scores path: bf16's 8-bit mantissa gives score
errors ~0.05 which exp() amplifies to ~2e-2 output error; fp16 keeps it ~3e-3.
"""

from collections import deque
from contextlib import ExitStack

import numpy as np

import concourse.bacc as bacc
import concourse.mybir as mybir
import concourse.tile as tile
from concourse.bass_utils import run_bass_kernel_spmd

F32 = mybir.dt.float32
F16 = mybir.dt.float16
BF16 = mybir.dt.bfloat16

D = 1024          # d_model
HD = 64           # head dim
NH_CORE = 8       # heads per core
OC = NH_CORE * HD # per-core q/k/v output dims (512)
N_CORES = 8
NI = D // 128     # contraction chunks for projections
NOC = OC // 128   # o-chunks (head pairs)
NDC = D // 128    # output-dim chunks for the final projection


def build_kernel(S=2048):
    nc = bacc.Bacc("TRN2", target_bir_lowering=False, debug=False)

    xq_d = nc.dram_tensor("xqT", (D, S), F16, kind="ExternalInput")
    xk_d = nc.dram_tensor("xkT", (D, S), F16, kind="ExternalInput")
    xv_d = nc.dram_tensor("xvT", (D, S), F16, kind="ExternalInput")
    wq_d = nc.dram_tensor("WqT", (D, OC), F16, kind="ExternalInput")
    wk_d = nc.dram_tensor("WkT", (D, OC), F16, kind="ExternalInput")
    wv_d = nc.dram_tensor("WvT", (D, OC), F16, kind="ExternalInput")
    wo_d = nc.dram_tensor("WoT", (OC, D), F16, kind="ExternalInput")
    bq_d = nc.dram_tensor("bq", (OC,), F32, kind="ExternalInput")
    bk_d = nc.dram_tensor("bk", (OC,), F32, kind="ExternalInput")
    y_d = nc.dram_tensor("yT", (D, S), BF16, kind="ExternalOutput")

    NKC = S // 128            # key chunks (16)
    QQ = 512                  # query-quarter size (s1 moving width per head)
    NQQ = S // QQ             # 4
    SH = S // 2               # xv half-slab width

    Exp = mybir.ActivationFunctionType.Exp
    Mult = mybir.AluOpType.mult

    with tile.TileContext(nc) as tc, ExitStack() as ctx:
        wpool = ctx.enter_context(tc.tile_pool(name="w", bufs=1))
        xpool = ctx.enter_context(tc.tile_pool(name="x", bufs=1))
        spool = ctx.enter_context(tc.tile_pool(name="seq", bufs=1))
        epool = ctx.enter_context(tc.tile_pool(name="e", bufs=3))
        evpool = ctx.enter_context(tc.tile_pool(name="ev", bufs=2))
        npool = ctx.enter_context(tc.tile_pool(name="nrm", bufs=2))
        pjpool = ctx.enter_context(tc.tile_pool(name="pj", bufs=1, space="PSUM"))
        s1pool = ctx.enter_context(tc.tile_pool(name="s1", bufs=2, space="PSUM"))
        accpool = ctx.enter_context(tc.tile_pool(name="acc", bufs=2, space="PSUM"))

        # ---- resident weights / biases / x slabs ----
        wq_sb = wpool.tile([128, NI, OC], F16, tag="wq")
        wk_sb = wpool.tile([128, NI, OC], F16, tag="wk")
        wv_sb = wpool.tile([128, NI, OC], F16, tag="wv")
        wo_sb = wpool.tile([128, NOC, D], F16, tag="wo")
        bq_sb = wpool.tile([128, NOC], F32, tag="bq")
        bk_sb = wpool.tile([128, NOC], F32, tag="bk")

        xq_sb = xpool.tile([128, NI, S], F16, tag="xq")
        xk_sb = xpool.tile([128, NI, S], F16, tag="xk")
        # xv streams through a 2-deep ring of quarter-slabs (512 t each);
        # the ring throttles the sync queue to a bounded lookahead and the
        # v-proj subunits consume quarters in order
        xv_sb = [xpool.tile([128, NI, S // 4], F16, tag="xv", bufs=2,
                            name=f"xvq{vq}") for vq in range(4)]

        # sync HW queue: xv half-slabs (2KB rows, v-proj chase), then xq
        # quarters 0/1 interleaved ic-major (q(hp0) chase)
        QV = S // 4

        def xv_dma(vq):
            for ic in range(NI):
                nc.sync.dma_start(
                    out=xv_sb[vq][:, ic, :],
                    in_=xv_d.ap()[ic * 128:(ic + 1) * 128, vq * QV:(vq + 1) * QV],
                )

        def xq_dma(qq):
            for ic in range(NI):
                nc.sync.dma_start(
                    out=xq_sb[:, ic, qq * QQ:(qq + 1) * QQ],
                    in_=xq_d.ap()[ic * 128:(ic + 1) * 128, qq * QQ:(qq + 1) * QQ],
                )

        # sync HW queue, in consumption order
        xv_dma(0)
        xq_dma(0)
        xv_dma(1)
        xq_dma(1)
        xv_dma(2)
        xv_dma(3)
        # scalar HW queue: wk + xk in quarter chunks so k-proj(hp0) can chase
        # quarter-by-quarter (first exp ~20us instead of waiting all of xk)
        nc.scalar.dma_start(out=wk_sb, in_=wk_d.ap().rearrange("(ic p) o -> p ic o", p=128))
        for tq in range(NQQ):
            for ic in range(NI):
                nc.scalar.dma_start(
                    out=xk_sb[:, ic, tq * QQ:(tq + 1) * QQ],
                    in_=xk_d.ap()[ic * 128:(ic + 1) * 128, tq * QQ:(tq + 1) * QQ],
                )
        # gpsimd SW queue: wv (first v matmul ~4us), wq, biases, xq 2/3, Wo
        nc.gpsimd.dma_start(out=wv_sb, in_=wv_d.ap().rearrange("(ic p) o -> p ic o", p=128))
        nc.gpsimd.dma_start(out=wq_sb, in_=wq_d.ap().rearrange("(ic p) o -> p ic o", p=128))
        nc.gpsimd.dma_start(out=bq_sb, in_=bq_d.ap().rearrange("(c p) -> p c", p=128))
        nc.gpsimd.dma_start(out=bk_sb, in_=bk_d.ap().rearrange("(c p) -> p c", p=128))
        for ic in range(NI):
            nc.gpsimd.dma_start(
                out=xq_sb[:, ic, 2 * QQ:4 * QQ],
                in_=xq_d.ap()[ic * 128:(ic + 1) * 128, 2 * QQ:4 * QQ],
            )
        nc.gpsimd.dma_start(out=wo_sb, in_=wo_d.ap().rearrange("(oc p) d -> p oc d", p=128))

        # ---- per-sequence slabs ----
        v_sb = spool.tile([128, NKC, NH_CORE * (HD + 1)], BF16, tag="v")
        qT_sb = spool.tile([128, NOC, S], F16, tag="qT")
        kT_sb = spool.tile([128, NOC, S], F16, tag="kT")
        # bf16 att: the y projection's N=1024 moving operand requires a
        # 16-bit dtype with 1024-wide ISA support (fp16 caps at 512)
        att_sb = spool.tile([128, NOC, S], BF16, tag="att")

        # ---- microtask unit factories (units are atomic: each holds the
        # ---- single [128,1024] pj psum slot until its trailing DVE op) ----
        def v_sub(vq, tp):
            """v projection for two key chunks (256 t): 16 MMs + copy."""
            kc0 = vq * 4 + tp * 2
            state = {}
            def mk_mm(ic):
                def f():
                    if "pj" not in state:
                        state["pj"] = pjpool.tile([128, 2 * OC], F32, tag="pj",
                                                  name=f"vps{vq}_{tp}")
                    for j in range(2):
                        nc.tensor.matmul(
                            state["pj"][:, j * OC:(j + 1) * OC],
                            xv_sb[vq][:, ic, (tp * 2 + j) * 128:(tp * 2 + j + 1) * 128],
                            wv_sb[:, ic, :],
                            start=(ic == 0), stop=(ic == NI - 1),
                        )
                return f
            def fin():
                for j in range(2):
                    vv = v_sb[:, kc0 + j, :].rearrange("p (h c) -> p h c", h=NH_CORE)
                    nc.vector.tensor_copy(
                        out=vv[:, :, 0:HD],
                        in_=state["pj"][:, j * OC:(j + 1) * OC]
                            .rearrange("p (h c) -> p h c", c=HD),
                    )
                    nc.vector.memset(vv[:, :, HD:HD + 1], 1.0)
            return [mk_mm(ic) for ic in range(NI)] + [fin]

        def qk_unit(w_sb, x_sb, b_sb, dst, hp, half, label):
            """q/k projection for two quarters (fp16 moving caps at N=512,
            so two MMs per stationary into the wide pj tile's halves)."""
            state = {}
            def mk_mm(ic):
                def f():
                    if "pj" not in state:
                        state["pj"] = pjpool.tile([128, 2 * QQ], F32, tag="pj",
                                                  name=f"pj{label}{hp}_{half}")
                    for j in range(2):
                        q0 = (half * 2 + j) * QQ
                        nc.tensor.matmul(
                            state["pj"][:, j * QQ:(j + 1) * QQ],
                            w_sb[:, ic, hp * 128:(hp + 1) * 128],
                            x_sb[:, ic, q0:q0 + QQ],
                            start=(ic == 0), stop=(ic == NI - 1),
                        )
                return f
            def bias():
                nc.vector.tensor_scalar_add(
                    out=dst[:, hp, half * 2 * QQ:(half + 1) * 2 * QQ],
                    in0=state["pj"][:, :],
                    scalar1=b_sb[:, hp:hp + 1],
                )
            return [mk_mm(ic) for ic in range(NI)] + [bias]

        def kq_quarter(w_sb, x_sb, b_sb, dst, hp, tq, label):
            """k/q projection for a single 512-col quarter (chase path)."""
            state = {}
            def mk_mm(icp):
                def f():
                    if "pj" not in state:
                        state["pj"] = pjpool.tile([128, 2 * QQ], F32, tag="pj",
                                                  name=f"pj1{label}{hp}_{tq}")
                    for ic in (2 * icp, 2 * icp + 1):
                        nc.tensor.matmul(
                            state["pj"][:, 0:QQ],
                            w_sb[:, ic, hp * 128:(hp + 1) * 128],
                            x_sb[:, ic, tq * QQ:(tq + 1) * QQ],
                            start=(ic == 0), stop=(ic == NI - 1),
                        )
                return f
            def bias():
                nc.vector.tensor_scalar_add(
                    out=dst[:, hp, tq * QQ:(tq + 1) * QQ],
                    in0=state["pj"][:, 0:QQ],
                    scalar1=b_sb[:, hp:hp + 1],
                )
            return [mk_mm(i) for i in range(NI // 2)] + [bias]

        def y_unit(dc, q0, qw, oc_mts, name):
            """output projection for [128 d rows, qw queries] (qw <= 1024 as
            1-2 N=512 MMs per stationary): oc-grouped MM microtasks (so the
            hp3-dependent oc can go last) + bf16 store."""
            nj = qw // QQ
            state = {}
            def mk_mm(ocs):
                def f():
                    if "pj" not in state:
                        state["pj"] = pjpool.tile([128, 2 * QQ], F32, tag="pj",
                                                  name=f"yps{name}_{dc}")
                    for oc in ocs:
                        for j in range(nj):
                            nc.tensor.matmul(
                                state["pj"][:, j * QQ:(j + 1) * QQ],
                                wo_sb[:, oc, dc * 128:(dc + 1) * 128],
                                att_sb[:, oc, q0 + j * QQ:q0 + (j + 1) * QQ],
                                start=(oc == 0), stop=(oc == NOC - 1),
                            )
                return f
            def store():
                y_sb = evpool.tile([128, 2 * QQ], BF16, tag="yev",
                                   name=f"yev{name}_{dc}")
                nc.vector.tensor_copy(out=y_sb[:, 0:qw], in_=state["pj"][:, 0:qw])
                nc.sync.dma_start(
                    out=y_d.ap()[dc * 128:(dc + 1) * 128, q0:q0 + qw],
                    in_=y_sb[:, 0:qw],
                )
            return [mk_mm(ocs) for ocs in oc_mts] + [store]

        # ---- filler schedule ----
        fillers = deque()

        def pull(n=1):
            for _ in range(n):
                if fillers:
                    fillers.popleft()()

        def run_unit(u):
            for f in u:
                f()

        # ---- prologue: chase the first DMA arrivals, minimal gate ----
        # wv+xv q0 -> v(kc0-3); xq qq0+wq -> q(0,qq0); wk+xk tq0 -> k(0,tq0).
        # First s1 issues ~20us in; block 0 then runs DMA-paced with the
        # rest of v proj and k(0,tq1-3) dripped in as deadline-ordered filler.
        run_unit(v_sub(0, 0))
        run_unit(v_sub(0, 1))
        run_unit(v_sub(1, 0))
        run_unit(kq_quarter(wk_sb, xk_sb, bk_sb, kT_sb, 0, 0, "k"))
        run_unit(kq_quarter(wq_sb, xq_sb, bq_sb, qT_sb, 0, 0, "q"))

        # ---- filler population in need-order ----
        # CORRECTNESS: a filler feeding block N must be fully emitted before
        # block N's first reader.  Pulls: block0 5/kc (80), block1 2/kc (32),
        # then 1/kc; deadlines verified against those budgets.
        fillers.extend(kq_quarter(wk_sb, xk_sb, bk_sb, kT_sb, 0, 1, "k"))
        fillers.extend(v_sub(1, 1))
        fillers.extend(kq_quarter(wk_sb, xk_sb, bk_sb, kT_sb, 0, 2, "k"))
        fillers.extend(v_sub(2, 0))
        fillers.extend(v_sub(2, 1))
        fillers.extend(kq_quarter(wk_sb, xk_sb, bk_sb, kT_sb, 0, 3, "k"))
        fillers.extend(v_sub(3, 0))
        fillers.extend(v_sub(3, 1))
        fillers.extend(kq_quarter(wq_sb, xq_sb, bq_sb, qT_sb, 0, 1, "q"))
        for hp in (1, 2, 3):
            fillers.extend(qk_unit(wk_sb, xk_sb, bk_sb, kT_sb, hp, 0, "k"))
            fillers.extend(qk_unit(wk_sb, xk_sb, bk_sb, kT_sb, hp, 1, "k"))
            fillers.extend(kq_quarter(wq_sb, xq_sb, bq_sb, qT_sb, hp, 0, "q"))
            fillers.extend(kq_quarter(wq_sb, xq_sb, bq_sb, qT_sb, hp, 1, "q"))
        fillers.extend(qk_unit(wq_sb, xq_sb, bq_sb, qT_sb, 0, 1, "q"))

        later_q = []
        for hp in (1, 2, 3):
            later_q.append(qk_unit(wq_sb, xq_sb, bq_sb, qT_sb, hp, 1, "q"))

        # block order: quarters 0/1 per pair first (k/q prefetch lead), then
        # quarters 2/3 swept by-quarter so only y(qq3) tails
        order = ([(hp, qq) for hp in range(NOC) for qq in (0, 1)]
                 + [(hp, 2) for hp in range(NOC)]
                 + [(hp, 3) for hp in range(NOC)])

        # ---- flat attention pipeline ----
        pend = deque()
        accs_by_bi = {}
        qq_done = {qq: 0 for qq in range(NQQ)}

        def finalize(bi, hp, qq):
            # copy accs to SBUF first: the copies free the acc PSUM ring
            # slots ~4us earlier than normalizing from PSUM would (the
            # recip/broadcast chain is long), so the next block's s2 never
            # stalls on the acc banks
            qoff = qq * QQ
            accs = accs_by_bi.pop(bi)
            asbs = []
            for hl in range(2):
                asb = npool.tile([65, QQ], F32, tag="accsb", bufs=2,
                                 name=f"asb{bi}_{hl}")
                nc.vector.tensor_copy(out=asb[:, :], in_=accs[hl][:, :])
                asbs.append(asb)
            for hl in range(2):
                off = hl * 64
                asb = asbs[hl]
                rt = npool.tile([1, QQ], F32, tag="rtmp", bufs=2,
                                name=f"rt{bi}_{hl}")
                nc.vector.tensor_copy(out=rt[:, :], in_=asb[64:65, :])
                nc.vector.reciprocal_approx_fast(out=rt[:, :], in_=rt[:, :])
                bc = npool.tile([64, QQ], F32, tag="bcast", bufs=2,
                                name=f"bc{bi}_{hl}")
                nc.gpsimd.partition_broadcast(out_ap=bc[:, :], in_ap=rt[:, :])
                nc.vector.tensor_tensor(
                    out=att_sb[off:off + 64, hp, qoff:qoff + QQ],
                    in0=asb[0:64, :],
                    in1=bc[:, :],
                    op=Mult,
                )
            qq_done[qq] += 1
            if qq_done[qq] == NOC:
                if qq == 1:
                    # quarters 0+1 done: y over N=1024 (halved ldweights)
                    for dc in range(NDC):
                        fillers.extend(
                            y_unit(dc, 0, 2 * QQ, [(0, 1), (2, 3)], "y01"))
                elif qq == 2:
                    for dc in range(NDC):
                        fillers.extend(
                            y_unit(dc, 2 * QQ, QQ, [(0, 1), (2, 3)], "y2"))
                # qq3 is emitted in the tail with oc3 last (see below)

        def s2_pop():
            bi2, hp2, qq2, e2, kc2 = pend.popleft()
            if kc2 == 0:
                accs_by_bi[bi2] = [
                    accpool.tile([65, QQ], F32, tag="acc", name=f"acc{bi2}_{hl}")
                    for hl in range(2)
                ]
            accs = accs_by_bi[bi2]
            for hl in range(2):
                h = 2 * hp2 + hl
                nc.tensor.matmul(
                    accs[hl][:, :],
                    v_sb[:, kc2, h * (HD + 1):(h + 1) * (HD + 1)],
                    e2[:, hl * QQ:(hl + 1) * QQ],
                    start=(kc2 == 0), stop=(kc2 == NKC - 1),
                )
            if kc2 == NKC - 1:
                finalize(bi2, hp2, qq2)

        for bi, (hp, qq) in enumerate(order):
            if bi == 6:
                for u in later_q:
                    fillers.extend(u)
            qoff = qq * QQ
            for kc in range(NKC):
                s1 = s1pool.tile([128, 2 * QQ], F32, tag="s1",
                                 name=f"s1_{bi}_{kc}")
                # the two heads' score MMs land on row groups 0/64 and
                # stream concurrently; one exp covers the fused tile
                for hl in range(2):
                    off = hl * 64
                    nc.tensor.matmul(
                        s1[:, hl * QQ:(hl + 1) * QQ],
                        kT_sb[off:off + 64, hp, kc * 128:(kc + 1) * 128],
                        qT_sb[off:off + 64, hp, qoff:qoff + QQ],
                        start=True, stop=True,
                    )
                e = epool.tile([128, 2 * QQ], BF16, tag="e", name=f"e{bi}_{kc}")
                nc.scalar.activation(out=e[:, :], in_=s1[:, :], func=Exp)
                pend.append((bi, hp, qq, e, kc))
                if len(pend) > 2:
                    s2_pop()
                # blocks 0/1 are DMA-paced; use the slack to pull the rest
                # of v proj and the k/q prefetch chain through
                pull(5 if bi == 0 else (2 if bi == 1 else 1))
        while pend:
            s2_pop()

        # tail: y(qq3) with the hp3-dependent oc last so most of the
        # projection overlaps the final block's normalize
        for dc in range(NDC):
            run_unit(y_unit(dc, 3 * QQ, QQ, [(0, 1), (2,), (3,)], "y3"))
        while fillers:
            fillers.popleft()()

    nc.compile()
    return nc


def make_in_maps(query, key, value, Wq, bq, Wk, bk, Wv, bv, Wo, bo):
    """Shard + lay out full inputs for the 8 cores: core = 2*n + g."""
    f16 = np.float16
    N = query.shape[0]
    per_g = {}
    for g in range(2):
        osl = slice(g * OC, (g + 1) * OC)
        per_g[g] = dict(
            WqT=np.ascontiguousarray(Wq[osl, :].T).astype(f16),
            WkT=np.ascontiguousarray(Wk[osl, :].T).astype(f16),
            WvT=np.ascontiguousarray(Wv[osl, :].T).astype(f16),
            WoT=np.ascontiguousarray(Wo[:, osl].T).astype(f16),
            bq=np.ascontiguousarray(bq[osl]).astype(np.float32),
            bk=np.ascontiguousarray(bk[osl]).astype(np.float32),
        )
    in_maps = []
    for n in range(N):
        xqT = np.ascontiguousarray(query[n].T).astype(f16)
        xkT = np.ascontiguousarray(key[n].T).astype(f16)
        xvT = np.ascontiguousarray(value[n].T).astype(f16)
        for g in range(2):
            m = dict(xqT=xqT, xkT=xkT, xvT=xvT)
            m.update(per_g[g])
            in_maps.append(m)
    return in_maps


_BUILT = None


def _get_built():
    global _BUILT
    if _BUILT is None:
        _BUILT = build_kernel(2048)
    return _BUILT


def kernel(query, key, value, Wq, bq, Wk, bk, Wv, bv, Wo, bo, _results=None):
    query = np.asarray(query, np.float32)
    key = np.asarray(key, np.float32)
    value = np.asarray(value, np.float32)
    Wq, bq = np.asarray(Wq, np.float32), np.asarray(bq, np.float32)
    Wk, bk = np.asarray(Wk, np.float32), np.asarray(bk, np.float32)
    Wv, bv = np.asarray(Wv, np.float32), np.asarray(bv, np.float32)
    Wo, bo = np.asarray(Wo, np.float32), np.asarray(bo, np.float32)

    N, S, _ = query.shape
    if _results is None:
        nc = _get_built()
        in_maps = make_in_maps(query, key, value, Wq, bq, Wk, bk, Wv, bv, Wo, bo)
        res = run_bass_kernel_spmd(nc, in_maps, list(range(N_CORES)))
        _results = res.results

    const = bv @ Wo.T + bo  # host-folded bias terms
    out = np.empty((N, S, D), np.float32)
    for n in range(N):
        yT = (_results[2 * n]["yT"].astype(np.float32)
              + _results[2 * n + 1]["yT"].astype(np.float32))
        out[n] = yT.T + const
    return out
